# revision 33
# baseline (speedup 1.0000x reference)
"""GPT-2 attention (B=2, S=2048, D=1024, H=16) on 8 TRN2 NeuronCores.

Sharding: 2-way data parallel over batch x 4-way tensor parallel over heads.
Core c handles batch b = c // 4 and heads 4g..4g+3 where g = c % 4.

Per-core kernel (all matmul inputs bf16, fp32 PSUM accumulation):
  1. QKV^T projection: Q^T, K^T computed in [head_dim, seq] layout
     (lhsT = W tiles, rhs = x^T tiles); V computed in natural [seq, head_dim]
     layout with denominator columns appended (ones col at 64 for even heads,
     at 65 for odd heads so each parity's softmax sum lands on its own PSUM
     partition). Inputs are host-repacked to the exact SBUF layouts so every
     DMA is a contiguous burst, and issued in first-use order with the first
     x/w pieces split fine so the first matmul starts ~1us in.
  2. Per (chunk, head) unit: scores^T[sk, sq] = K^T_tile.T @ Q^T (one K=64
     matmul per sk tile, restricted to valid causal columns), exp on ScalarE
     (scale = 1/sqrt(64)) directly PSUM->SBUF bf16, then the causal mask is
     applied by zeroing the strict upper triangle of the diagonal block on
     the (otherwise idle) Pool engine - no PE mask matmuls.
  3. AV: O_aug^T[66, sq] = V_aug.T @ P^T accumulated over sk tiles; row 64
     (even heads) / 65 (odd heads) is the softmax denominator.
  4. Normalization: per head the denominator row is reciprocal'd in place on
     DVE (single lane) into a shared per-(chunk, head-pair) tile; once both
     parities are in, a single K=2 selector matmul broadcasts both recips
     across the pair's 128 partitions and one DVE multiply normalizes both
     heads' O^T at once.
  5. Output projection: y_partial[sq, 1024] = O_scaled^T.T @ Wproj_shard.
     The first 3/4 (which only need already-normalized chunks) are emitted
     before the final norms to cover their latency.

Host: inputs pre-cast to bf16 and pre-repacked per core; the 4 per-batch
partials are summed on host (row-split matmul unshard) and bproj added.
"""

import numpy as np
import ml_dtypes

import concourse.bass as bass
import concourse.mybir as mybir
import concourse.tile as tile
from concourse import bacc
from concourse.bass_utils import run_bass_kernel_spmd

BF16 = ml_dtypes.bfloat16

B, S, D, H = 2, 2048, 1024, 16
HD = D // H            # 64
NH = 4                 # heads per core
JQ = NH * HD           # 256 q (or k, or v) columns per core
P = 128
SC = 512               # seq chunk (matmul free dim / PSUM bank)
NSC = S // SC          # 4
NST = S // P           # 16 seq tiles
NDC = D // P           # 8 contraction chunks over model dim
SCALE = 1.0 / np.sqrt(np.float32(HD))  # 0.125

# block index of each QKV 128-col group in the host-repacked w
# (blocks [K0, K1, Q0, Q1, V0, V1]); jt 0,1 -> Q halves; jt 2,3 -> K halves
WBLK = {2: 0, 3: 1, 0: 2, 1: 3}

_COMPILED = {}


def build(has_qkv_bias: bool, reps: int = 1):
    # reps > 1 unrolls the whole body N times inside one NEFF - a benchmark
    # aid: (T(reps=K) - T(reps=1)) / (K-1) isolates per-iteration HW time
    # from host/tunnel dispatch overhead. Production path uses reps=1.
    f32 = mybir.dt.float32
    bf16 = mybir.dt.bfloat16
    nc = bacc.Bacc()

    xp = nc.declare_dram_parameter("xp", [P, NSC, NDC, SC], bf16, isOutput=False)
    # w blocks: dim1 = [K0, K1, Q0, Q1, V0, V1] so each (block, a-range) DMA
    # piece is a contiguous >=512B burst
    w = nc.declare_dram_parameter("w", [P, 6, NDC, P], bf16, isOutput=False)
    wp = nc.declare_dram_parameter("wp", [P, JQ // P, D], bf16, isOutput=False)
    if has_qkv_bias:
        bqkv = nc.declare_dram_parameter("bqkv", [1, 6 * P], bf16, isOutput=False)
    y = nc.declare_dram_parameter("y", [S, D], bf16, isOutput=True)

    with tile.TileContext(nc) as tc:
        with (
            tc.tile_pool(name="const", bufs=1) as const,
            tc.tile_pool(name="inp", bufs=1) as inp,
            tc.tile_pool(name="qkv", bufs=1) as qkv,
            tc.tile_pool(name="ptp", bufs=5) as ptp,
            tc.tile_pool(name="ps", bufs=4, space="PSUM") as psp,
        ):
          for _rep in range(reps):
            # ---- constants ----
            ones = const.tile([P, SC], bf16)
            nc.gpsimd.memset(ones[:], 1.0)
            # K=2 selector: broadcasts recip row 64 to partitions 0-63 and
            # row 65 to partitions 64-127 in one matmul
            sel2 = const.tile([P, P], bf16)
            nc.gpsimd.memset(sel2[:], 0.0)
            nc.gpsimd.memset(sel2[64:65, 0:64], 1.0)
            nc.gpsimd.memset(sel2[96:97, 64:128], 1.0)

            # ---- load inputs, split so early consumers start immediately ----
            # wsb blocks: [K0, K1, Q0, Q1, V0, V1]; jt -> block via WBLK
            wsb = inp.tile([P, 6, NDC, P], bf16)
            xc = [inp.tile([P, NDC, SC], bf16, name=f"xc{c}") for c in range(NSC)]

            def dma_w(blk, a0=0, a1=NDC):
                nc.sync.dma_start(
                    out=wsb[:, blk, a0:a1, :], in_=w[:, blk, a0:a1, :]
                )

            def dma_x(c, a0=0, a1=NDC):
                nc.sync.dma_start(
                    out=xc[c][:, a0:a1, :], in_=xp[:, c, a0:a1, :]
                )

            # first-use order, first pieces fine-grained: the interleaved
            # K/Q chunk-0 production consumes each (w, x) a-pair as it lands
            dma_w(0, 0, 2)             # K0 a=0,1
            dma_x(0, 0, 2)
            dma_w(2, 0, 2)             # Q0 a=0,1
            dma_x(0, 2, 4)
            dma_w(0, 2, NDC)           # K0 rest
            dma_w(2, 2, NDC)           # Q0 rest
            dma_x(0, 4, 6)
            dma_x(0, 6, NDC)
            dma_w(4)                   # V0
            dma_w(5)                   # V1
            dma_w(1)                   # K1
            dma_w(3)                   # Q1
            if has_qkv_bias:
                b_sb = inp.tile([1, 6 * P], bf16)
                nc.sync.dma_start(out=b_sb[:], in_=bqkv[:])
            dma_x(1)
            dma_x(2)
            wp_sb = inp.tile([P, JQ // P, D], bf16)
            nc.sync.dma_start(out=wp_sb[:], in_=wp[:])
            dma_x(3)

            qT = qkv.tile([P, 2, S], bf16)  # partitions: head pair (h%2)*64 + hd
            kT = qkv.tile([P, 2, S], bf16)
            # V with two denominator columns: col 64 = 1 for even heads,
            # col 96 = 1 for odd heads (96 is a legal DVE start partition, so
            # each parity's denominator recip stays lane-aligned; 65-95 are
            # zero padding)
            v_sb = qkv.tile([P, NST, NH, 97], bf16)
            nc.vector.memset(v_sb[:, :, :, HD:97], 0.0)
            for h in range(NH):
                col = HD if h % 2 == 0 else 96
                nc.vector.memset(v_sb[:, :, h, col:col + 1], 1.0)
            oT = qkv.tile([P, 2, S], bf16)

            def emit_qk_chunk(jt, c):
                # one [128, SC] chunk of Q^T (jt 0,1) or K^T (jt 2,3)
                dest, jl = (qT, jt) if jt < 2 else (kT, jt - 2)
                blk = WBLK[jt]
                ps_qkv = psp.tile([P, SC], f32, tag="ps", name="ps_qkv")
                for a in range(NDC):
                    nc.tensor.matmul(
                        ps_qkv[:],
                        lhsT=wsb[:, blk, a, :],
                        rhs=xc[c][:, a, :],
                        start=(a == 0),
                        stop=(a == NDC - 1) and not has_qkv_bias,
                    )
                if has_qkv_bias:
                    nc.tensor.matmul(
                        ps_qkv[:],
                        lhsT=b_sb[0:1, blk * P:(blk + 1) * P],
                        rhs=ones[0:1, :SC],
                        start=False,
                        stop=True,
                    )
                nc.vector.tensor_copy(dest[:, jl, c * SC:(c + 1) * SC], ps_qkv[:])

            def emit_qk0_interleaved():
                # chunk-0 K0/Q0 production interleaved a-piece by a-piece so
                # the PE consumes each x DMA piece the moment it lands
                ps_k = psp.tile([P, SC], f32, tag="ps", name="ps_k0")
                ps_q = psp.tile([P, SC], f32, tag="ps", name="ps_q0")
                for a in range(NDC):
                    last = (a == NDC - 1) and not has_qkv_bias
                    nc.tensor.matmul(
                        ps_k[:], lhsT=wsb[:, 0, a, :], rhs=xc[0][:, a, :],
                        start=(a == 0), stop=last,
                    )
                    nc.tensor.matmul(
                        ps_q[:], lhsT=wsb[:, 2, a, :], rhs=xc[0][:, a, :],
                        start=(a == 0), stop=last,
                    )
                if has_qkv_bias:
                    nc.tensor.matmul(
                        ps_k[:], lhsT=b_sb[0:1, 0:P], rhs=ones[0:1, :SC],
                        start=False, stop=True,
                    )
                    nc.tensor.matmul(
                        ps_q[:], lhsT=b_sb[0:1, 2 * P:3 * P], rhs=ones[0:1, :SC],
                        start=False, stop=True,
                    )
                nc.vector.tensor_copy(kT[:, 0, 0:SC], ps_k[:])
                nc.vector.tensor_copy(qT[:, 0, 0:SC], ps_q[:])

            def emit_v_tile(t):
                # V rows for seq tile t, all 4 heads
                ps_v = psp.tile([P, SC], f32, tag="ps", name="ps_v")
                for a in range(NDC):
                    nc.tensor.matmul(
                        ps_v[:, 0:JQ],
                        lhsT=xc[t // 4][:, a, (t % 4) * P:(t % 4 + 1) * P],
                        rhs=wsb[:, 4:6, a, :],
                        start=(a == 0),
                        stop=(a == NDC - 1) and not has_qkv_bias,
                    )
                if has_qkv_bias:
                    nc.tensor.matmul(
                        ps_v[:, 0:JQ],
                        lhsT=ones[0:1, 0:P],
                        rhs=b_sb[0:1, 4 * P:6 * P],
                        start=False,
                        stop=True,
                    )
                nc.vector.tensor_copy(
                    v_sb[:, t, :, 0:HD],
                    ps_v[:, 0:JQ].rearrange("p (h d) -> p h d", d=HD),
                )

            # per-(chunk, head-pair) normalization: both parities' recips
            # land in one tile (rows 64/65), then a K=2 selector matmul
            # broadcasts both across 128 partitions and one DVE multiply
            # normalizes both heads' O^T
            pair_stash = {}
            norm_pend = []

            def emit_norm():
                c, jl, recip2 = norm_pend.pop(0)
                ps_bc = psp.tile([P, SC], f32, tag="ps", name="ps_bc")
                nc.tensor.matmul(
                    ps_bc[:],
                    lhsT=sel2[64:97, :],
                    rhs=recip2[64:97, :],
                    start=True,
                    stop=True,
                )
                nc.vector.tensor_mul(
                    oT[:, jl, c * SC:(c + 1) * SC],
                    oT[:, jl, c * SC:(c + 1) * SC],
                    ps_bc[:],
                )

            # ---- interleaved scheduler ----
            # The PE is in-order and ScalarE needs ~2x the PE's (tiled)
            # time per scores step, so scores t-steps are interleaved at
            # emission time with queued background work (AV of the previous
            # pair, QKV production, projections, norms) - the PE never sits
            # behind a scores PSUM bank while other work exists.
            from collections import deque

            bq = deque()  # thunks, each emitting ~4 matmuls of filler work

            def drain(n=None):
                cnt = 0
                while bq and (n is None or cnt < n):
                    bq.popleft()()
                    cnt += 1

            def sched_unit(c, jl):
                # scores for both heads of pair jl of chunk c: the even head
                # (partitions 0-63) emits as PE tile T0 and the odd head
                # (64-127) as T8, so the two matmuls of each t-step land on
                # independent 64x128 row tiles and overlap on hardware. The
                # heads share one 2-bank PSUM tile per sk tile, so a single
                # exp (and, on diagonal tiles, one Pool mask) evacuates both.
                nv = min(4 * (c + 1), NST)  # valid sk tiles
                pt2 = ptp.tile([P, NST, 2, SC], bf16, tag="pt", name="pt",
                               bufs=3)
                for t in range(nv):
                    # first valid column within this sq chunk (causal)
                    coff = max(0, t * P - c * SC)
                    ps2 = psp.tile([P, 2, SC], f32, tag="ps_sc",
                                   name="ps_sc", bufs=2)
                    for par in range(2):
                        po = par * HD
                        nc.tensor.matmul(
                            ps2[:, par, coff:],
                            lhsT=kT[po:po + HD, jl, t * P:(t + 1) * P],
                            rhs=qT[po:po + HD, jl, c * SC + coff:(c + 1) * SC],
                            start=True,
                            stop=True,
                        )
                    nc.scalar.activation(
                        pt2[:, t, :, coff:], ps2[:, :, coff:],
                        mybir.ActivationFunctionType.Exp,
                        scale=float(SCALE),
                    )
                    if t >= 4 * c:
                        # causal mask: zero the strict upper triangle of the
                        # diagonal block (both heads) on the idle Pool engine
                        nc.gpsimd.affine_select(
                            out=pt2[:, t, :, coff:coff + P],
                            in_=pt2[:, t, :, coff:coff + P],
                            compare_op=mybir.AluOpType.is_ge,
                            fill=0.0,
                            base=0,
                            pattern=[[0, 2], [1, P]],
                            channel_multiplier=-1,
                        )
                    # spread the queued filler work across the unit
                    left = nv - 1 - t
                    if left > 0:
                        drain(max(1, len(bq) // (left + 1)))
                drain()
                return [(c, 2 * jl, pt2), (c, 2 * jl + 1, pt2)]

            def push_av(state, tail=False):
                # AV for one head, split into ~4-matmul thunks on the queue
                c, h, pt2 = state
                jl, po, par = h // 2, (h % 2) * HD, h % 2
                nv = min(4 * (c + 1), NST)
                mrows = 65 if h % 2 == 0 else 97  # odd heads: denom on row 96
                cell = {}

                def chunk(t0, t1, first, last):
                    if first:
                        cell["ps"] = psp.tile([P, SC], f32, tag="ps",
                                              name="ps_av")
                    ps_av = cell["ps"]
                    for t in range(t0, t1):
                        coff = max(0, t * P - c * SC)
                        nc.tensor.matmul(
                            ps_av[0:mrows, coff:],
                            lhsT=v_sb[:, t, h, 0:mrows],
                            rhs=pt2[:, t, par, coff:],
                            start=(t == 0),
                            stop=(t == nv - 1),
                        )
                    if not last:
                        return
                    if h % 2 == 0:
                        recip2 = ptp.tile([P, SC], bf16, tag="recip2",
                                          name="recip2", bufs=3)
                        # rows 65-95 are dead weight in the K=33 selector
                        # matmul; zero recycled-buffer garbage
                        nc.gpsimd.memset(recip2[64:96, :], 0.0)
                        pair_stash[(c, jl)] = recip2
                    else:
                        recip2 = pair_stash.pop((c, jl))
                    row = HD if h % 2 == 0 else 96
                    with nc.allow_low_precision(
                        reason="bf16 softmax denom recip"
                    ):
                        nc.vector.reciprocal(
                            recip2[row:row + 1, :], ps_av[row:row + 1, :]
                        )
                    if tail:  # ScalarE is idle after the last exp
                        nc.scalar.copy(
                            oT[po:po + HD, jl, c * SC:(c + 1) * SC],
                            ps_av[0:HD, :],
                        )
                    else:
                        nc.vector.tensor_copy(
                            oT[po:po + HD, jl, c * SC:(c + 1) * SC],
                            ps_av[0:HD, :],
                        )
                    if h % 2 == 1:
                        norm_pend.append((c, jl, recip2))

                for i in range(0, nv, 4):
                    bq.append(
                        lambda t0=i, t1=min(i + 4, nv), f=(i == 0),
                        l=(i + 4 >= nv): chunk(t0, t1, f, l)
                    )

            def emit_proj(st, jc):
                ps_y = psp.tile([P, SC], f32, tag="ps", name="ps_y")
                for cc in range(2):
                    nc.tensor.matmul(
                        ps_y[:],
                        lhsT=oT[:, cc, st * P:(st + 1) * P],
                        rhs=wp_sb[:, cc, jc * SC:(jc + 1) * SC],
                        start=(cc == 0),
                        stop=(cc == 1),
                    )
                y_sb = ptp.tile([P, SC], bf16, tag="ysb", name="y_sb", bufs=4)
                if st >= 12:
                    # tail tiles: both ScalarE and DVE are draining - split
                    # the evacuation across them so neither serializes
                    hc = SC // 2
                    nc.scalar.copy(y_sb[:, 0:hc], ps_y[:, 0:hc])
                    nc.vector.tensor_copy(y_sb[:, hc:], ps_y[:, hc:])
                else:
                    nc.vector.tensor_copy(y_sb[:], ps_y[:])
                nc.sync.dma_start(
                    out=y[st * P:(st + 1) * P, jc * SC:(jc + 1) * SC],
                    in_=y_sb[:],
                )

            # ---- queue-item constructors for background (filler) work ----
            def push_prod_qk(jt, c):
                dest, jl2 = (qT, jt) if jt < 2 else (kT, jt - 2)
                blk = WBLK[jt]
                cell = {}

                def half(a0, a1, first, last):
                    if first:
                        cell["ps"] = psp.tile([P, SC], f32, tag="ps",
                                              name="ps_qkv")
                    ps_qkv = cell["ps"]
                    for a in range(a0, a1):
                        nc.tensor.matmul(
                            ps_qkv[:],
                            lhsT=wsb[:, blk, a, :],
                            rhs=xc[c][:, a, :],
                            start=(a == 0),
                            stop=(a == NDC - 1) and not has_qkv_bias,
                        )
                    if not last:
                        return
                    if has_qkv_bias:
                        nc.tensor.matmul(
                            ps_qkv[:],
                            lhsT=b_sb[0:1, blk * P:(blk + 1) * P],
                            rhs=ones[0:1, :SC],
                            start=False,
                            stop=True,
                        )
                    nc.vector.tensor_copy(
                        dest[:, jl2, c * SC:(c + 1) * SC], ps_qkv[:]
                    )

                bq.append(lambda: half(0, 4, True, False))
                bq.append(lambda: half(4, NDC, False, True))

            def push_prod_v(t):
                cell = {}

                def half(a0, a1, first, last):
                    if first:
                        cell["ps"] = psp.tile([P, SC], f32, tag="ps",
                                              name="ps_v")
                    ps_v = cell["ps"]
                    for a in range(a0, a1):
                        nc.tensor.matmul(
                            ps_v[:, 0:JQ],
                            lhsT=xc[t // 4][:, a, (t % 4) * P:(t % 4 + 1) * P],
                            rhs=wsb[:, 4:6, a, :],
                            start=(a == 0),
                            stop=(a == NDC - 1) and not has_qkv_bias,
                        )
                    if not last:
                        return
                    if has_qkv_bias:
                        nc.tensor.matmul(
                            ps_v[:, 0:JQ],
                            lhsT=ones[0:1, 0:P],
                            rhs=b_sb[0:1, 4 * P:6 * P],
                            start=False,
                            stop=True,
                        )
                    nc.vector.tensor_copy(
                        v_sb[:, t, :, 0:HD],
                        ps_v[:, 0:JQ].rearrange("p (h d) -> p h d", d=HD),
                    )

                bq.append(lambda: half(0, 4, True, False))
                bq.append(lambda: half(4, NDC, False, True))

            def push_norms(climit):
                def f():
                    while norm_pend and norm_pend[0][0] <= climit:
                        emit_norm()

                bq.append(f)

            def push_proj(st, jc):
                bq.append(lambda: emit_proj(st, jc))

            # ---- main schedule ----
            # 8 pair-units k = 2c + jl. Each unit's scores interleave with
            # the queued AV of the previous pair, production for upcoming
            # chunks, and projections whose norms completed a chunk ago.
            emit_qk0_interleaved()

            # k=0: pair (0,0); filler: V0-3, K1/Q1 of chunk 0
            for t in range(4):
                push_prod_v(t)
            push_prod_qk(3, 0)
            push_prod_qk(1, 0)
            s00 = sched_unit(0, 0)

            # k=1: pair (0,1); filler: AV(0,0), all chunk-1 production
            for s in s00:
                push_av(s)
            push_prod_qk(2, 1)
            push_prod_qk(0, 1)
            for t in range(4, 8):
                push_prod_v(t)
            push_prod_qk(3, 1)
            push_prod_qk(1, 1)
            s01 = sched_unit(0, 1)

            # k=2: pair (1,0); filler: AV(0,1), chunk-2 K0/Q0
            for s in s01:
                push_av(s)
            push_prod_qk(2, 2)
            push_prod_qk(0, 2)
            s10 = sched_unit(1, 0)

            # k=3: pair (1,1); filler: AV(1,0), chunk-0 norms, chunk-2 rest
            for s in s10:
                push_av(s)
            push_norms(0)
            for t in range(8, 12):
                push_prod_v(t)
            push_prod_qk(3, 2)
            push_prod_qk(1, 2)
            s11 = sched_unit(1, 1)

            # k=4: pair (2,0); filler: AV(1,1), proj st0-1, chunk-3 K0/Q0
            for s in s11:
                push_av(s)
            for st in (0, 1):
                push_proj(st, 0)
                push_proj(st, 1)
            push_prod_qk(2, 3)
            push_prod_qk(0, 3)
            s20 = sched_unit(2, 0)

            # k=5: pair (2,1); filler: AV(2,0), chunk-1 norms, proj st2-3,
            # chunk-3 rest
            for s in s20:
                push_av(s)
            push_norms(1)
            for st in (2, 3):
                push_proj(st, 0)
                push_proj(st, 1)
            for t in range(12, 16):
                push_prod_v(t)
            push_prod_qk(3, 3)
            push_prod_qk(1, 3)
            s21 = sched_unit(2, 1)

            # k=6: pair (3,0); filler: AV(2,1), proj st4-7
            for s in s21:
                push_av(s)
            for st in (4, 5, 6, 7):
                push_proj(st, 0)
                push_proj(st, 1)
            s30 = sched_unit(3, 0)

            # k=7: pair (3,1); filler: AV(3,0), chunk-2 norms, proj st8-11
            for s in s30:
                push_av(s)
            push_norms(2)
            for st in (8, 9, 10, 11):
                push_proj(st, 0)
                push_proj(st, 1)
            s31 = sched_unit(3, 1)

            # tail: AV(3,1) (ScalarE evac - it is idle after the last exp),
            # chunk-3 norms, then the last projections
            for s in s31:
                push_av(s, tail=True)
            drain()
            while norm_pend:
                emit_norm()
            for st in range(12, NST):
                for jc in range(2):
                    emit_proj(st, jc)

    nc.compile()
    return nc


def get_compiled(has_qkv_bias: bool):
    key = bool(has_qkv_bias)
    if key not in _COMPILED:
        _COMPILED[key] = build(key)
    return _COMPILED[key]


def make_in_maps(x, Wqkv, bqkv, Wproj):
    has_bias = bool(np.any(bqkv))
    # x[b] [S, D] -> [128, NSC, NDC, SC]: xp[p, c, a, s] = x[b][c*SC+s, a*P+p]
    xps = [
        np.ascontiguousarray(
            x[b].reshape(NSC, SC, NDC, P).transpose(3, 0, 2, 1)
        ).astype(BF16)
        for b in range(B)
    ]
    in_maps = []
    for c in range(8):
        b, g = c // 4, c % 4
        sl = slice(g * JQ, (g + 1) * JQ)
        # column order K | Q | V to match first-use order in the kernel
        wshard = np.concatenate(
            [Wqkv[:, D + g * JQ:D + (g + 1) * JQ], Wqkv[:, sl],
             Wqkv[:, 2 * D + g * JQ:2 * D + (g + 1) * JQ]], axis=1
        )
        # [D, 768] -> [128, 6 blocks, NDC, 128]
        wshard = wshard.reshape(NDC, P, 6, P).transpose(1, 2, 0, 3)
        wpshard = Wproj[sl].reshape(JQ // P, P, D).transpose(1, 0, 2)
        m = {
            "xp": xps[b],
            "w": np.ascontiguousarray(wshard).astype(BF16),
            "wp": np.ascontiguousarray(wpshard).astype(BF16),
        }
        if has_bias:
            bshard = np.concatenate(
                [bqkv[D + g * JQ:D + (g + 1) * JQ], bqkv[sl],
                 bqkv[2 * D + g * JQ:2 * D + (g + 1) * JQ]]
            ).astype(BF16)
            m["bqkv"] = np.ascontiguousarray(bshard[None, :])
        in_maps.append(m)
    return has_bias, in_maps


def run(x, Wqkv, bqkv, Wproj, bproj, trace=False):
    has_bias, in_maps = make_in_maps(x, Wqkv, bqkv, Wproj)
    nc = get_compiled(has_bias)
    res = run_bass_kernel_spmd(nc, in_maps, core_ids=list(range(8)), trace=trace)
    out = np.zeros((B, S, D), np.float32)
    for c in range(8):
        out[c // 4] += res.results[c]["y"].astype(np.float32)
    out += bproj.astype(np.float32)
    return out, res


def kernel(x, Wqkv, bqkv, Wproj, bproj):
    x = np.asarray(x, np.float32)
    Wqkv = np.asarray(Wqkv, np.float32)
    bqkv = np.asarray(bqkv, np.float32)
    Wproj = np.asarray(Wproj, np.float32)
    bproj = np.asarray(bproj, np.float32)
    out, _ = run(x, Wqkv, bqkv, Wproj, bproj, trace=False)
    return out
